# revision 1
# baseline (speedup 1.0000x reference)
"""DOSAConLoss Trainium2 kernel.

result = mean(base) * mean(1 + ALPHA * density)
       = mean(base) * (1 + ALPHA * (N/1024) / max_hist)

since sum(hist) == N exactly (every box center lands in one bin).

Per core (8-way data parallel over N): compute
  - per-partition partial sums of base  (acc_out [128, n_tiles])
  - partial 32x32 histogram of target box centers (hist_out [32, 32])
Host combines: sums acc, sums hists (minus padding), applies the scalar formula.

Math rewrite (validated vs reference in fp64/fp32):
  dx=x1-x2, W=w1+w2, dW=w1-w2 (same for y/h)
  iw4 = relu(W - max(|2dx|,|dW|)) = 2*iw ; inter4 = iw4*ih4 = 4*inter
  union = a1+a2 - inter4/4 (+eps)   ; iou = inter4 * 0.25/(union+eps)
  cw2 = W + mx = 2*cw ; c24 = cw2^2+ch2^2 = 4*c2 ; rho4 = (2dx)^2+(2dy)^2
  rho2/c2 == rho4/c24
  atan(w/h) range-reduced: q~ = min(w,h)/max(w,h) in [0,1];
     theta = atan(q~) + [w>h]*(pi/2 - 2*atan(q~))
  v = ((th2-th1)*2/pi)^2 ; a = v/(v-iou+1+eps)
  ciou = iou - rho4/c24 - v^2/(v-iou+1+eps)
  base = (1-ciou)^3 / (w2*h2 + 1e-7)
Reciprocals via exp(-ln(x)) (ACT Reciprocal is disallowed in bass).
Histogram: floor(32*x) via magic-number RNE rounding (mod/divide are not
  ISA-legal on DVE); x-side: 32 bin-major bf16 one-hot tensor_scalar
  is_equal ops; y-side packed to 16 rows with radix-512 parity weights
  (uy[m] = [floor(16y)==m] * (1 + 511*(gy mod 2))), so TensorE accumulates
  psum[16,32] += uy[:,:,t].T @ ohx[:,:,t] per 128-box column, in 4
  accumulation groups of 2 tiles (cell counts stay < 512 for exact radix
  decode). Host decodes the packed groups, exactly relocating the ~1e-6
  fraction of fp-tie boxes where the device trick-bin differs from floor.
"""

import numpy as np

import concourse.bass as bass
import concourse.bacc as bacc
import concourse.mybir as mybir
import concourse.tile as tile
from concourse import bass_utils

# The act-table-load chooser picks the first set containing each function,
# which puts Ln in `natural_log` and Exp in `exp_and_others`, forcing a
# ~2.7us table switch at every Ln->Exp pair (we use exp(-ln(x)) for all
# reciprocals). Hide Ln/Exp from the single-function sets so the chooser
# lands on `natural_log_exp_and_others` (set ids keep their act_info.json
# positions; only membership is masked).
_orig_get_act_tables = bacc.get_activation_tables


def _patched_get_act_tables(arch):
    t = {k: set(v) for k, v in _orig_get_act_tables(arch).items()}
    t.get("natural_log", set()).discard(mybir.ActivationFunctionType.Ln)
    t.get("exp_and_others", set()).discard(mybir.ActivationFunctionType.Exp)
    t.get("exp_and_friends", set()).discard(mybir.ActivationFunctionType.Exp)
    return t


bacc.get_activation_tables = _patched_get_act_tables

F32 = mybir.dt.float32
BF16 = mybir.dt.bfloat16
AF = mybir.ActivationFunctionType
OP = mybir.AluOpType

GRID = 32
ALPHA = 1.5
EPS = 1e-7
PI = float(np.pi)
MAGIC = float(2 ** 23)

N_CORES = 8
N_TOTAL = 4_000_000
NB_CORE = 524_288            # padded boxes per core: 128 * 4096
PAD_BOX = (0.5, 0.5, 1.0, 1.0)  # pred==targ box -> base contribution ~1e-21, bin (16,16)

# GPSIMD offload set for 2-input tensor_tensor ops (tune via profile)
# (POOL TensorTensor float ops: only add/subtract/mult are ISA-legal)
GPS_OPS = {"asum", "cw2", "ch2", "c24", "rho4", "th2a", "th1a", "dat", "term2", "s12"}


def build_nc(NB, T=512, Tc=512, gps=True):
    """Build the per-core Bass program. NB must equal n_tiles*128*T."""
    n_tiles = NB // (128 * T)
    assert NB == n_tiles * 128 * T
    n_chunks = T // Tc
    assert T == n_chunks * Tc

    nc = bacc.Bacc("TRN2", target_bir_lowering=False, debug=False)
    pred_d = nc.dram_tensor("pred_boxes", [NB, 4], F32, kind="ExternalInput")
    targ_d = nc.dram_tensor("target_boxes", [NB, 4], F32, kind="ExternalInput")
    acc_d = nc.dram_tensor("acc_out", [128, n_tiles], F32, kind="ExternalOutput")
    n_grp_ = max(1, (NB // (128 * T)) // 2)
    hist_d = nc.dram_tensor("hist_out", [GRID // 2, GRID * n_grp_], F32, kind="ExternalOutput")

    pred_v = pred_d.ap().rearrange("(n p t) c -> n p (t c)", p=128, t=T)
    targ_v = targ_d.ap().rearrange("(n p t) c -> n p (t c)", p=128, t=T)

    def eng(name):
        return nc.gpsimd if (gps and name in GPS_OPS) else nc.vector

    with tile.TileContext(nc) as tc:
        with (
            tc.tile_pool(name="inp", bufs=3) as inp,
            tc.tile_pool(name="tmp", bufs=2) as tmp,
            tc.tile_pool(name="ohp", bufs=2) as ohp,
            tc.tile_pool(name="cst", bufs=1) as cst,
            tc.tile_pool(name="psp", bufs=1, space="PSUM") as psp,
        ):
            bias_tiles = {}

            def bias_ap(val):
                if val not in bias_tiles:
                    t = cst.tile([128, 1], F32, name=f"bias{len(bias_tiles)}")
                    nc.vector.memset(t[:], val)
                    bias_tiles[val] = t[:]
                return bias_tiles[val]
            acc_sb = cst.tile([128, n_tiles], F32)
            n_grp = max(1, n_tiles // 2)
            hist_sb = cst.tile([GRID // 2, GRID * n_grp], F32)
            ps_g = [psp.tile([GRID // 2, GRID], F32, name=f"ps{g}") for g in range(n_grp)]

            mm_i = 0
            total_mms = NB // 128

            # Temp slot allocator: long-lived temps get dedicated tags;
            # short-lived ones rotate through NGEN generic tags (bufs=2 each,
            # Tile inserts WAR deps on slot reuse). Max temp lifetime must be
            # < 2*NGEN generic allocations.
            NGEN = 12
            DEDICATED = {"a2t", "iou", "term1"}
            gen_counter = [0]

            for n in range(n_tiles):
                pt = inp.tile([128, 4 * T], F32, tag="pred")
                tt = inp.tile([128, 4 * T], F32, tag="targ")
                nc.sync.dma_start(pt[:], pred_v[n])
                nc.sync.dma_start(tt[:], targ_v[n])
                p3 = pt.rearrange("p (t c) -> p c t", c=4)
                t3 = tt.rearrange("p (t c) -> p c t", c=4)
                x1, y1, w1, h1 = p3[:, 0], p3[:, 1], p3[:, 2], p3[:, 3]
                x2, y2, w2, h2 = t3[:, 0], t3[:, 1], t3[:, 2], t3[:, 3]

                def t_(tag):
                    if tag in DEDICATED:
                        return tmp.tile([128, T], F32, tag=tag, name=tag)[:]
                    i = gen_counter[0] % NGEN
                    gen_counter[0] += 1
                    return tmp.tile([128, T], F32, tag=f"g{i}", name=tag)[:]

                dx, dy = t_("dx"), t_("dy")
                W, dW, H, dH = t_("W"), t_("dW"), t_("H"), t_("dH")
                nc.vector.tensor_tensor(dx, x1, x2, OP.subtract)
                nc.vector.tensor_tensor(dy, y1, y2, OP.subtract)
                nc.vector.tensor_tensor(W, w1, w2, OP.add)
                nc.vector.tensor_tensor(dW, w1, w2, OP.subtract)
                nc.vector.tensor_tensor(H, h1, h2, OP.add)
                nc.vector.tensor_tensor(dH, h1, h2, OP.subtract)
                a2t, a1t, asum = t_("a2t"), t_("a1t"), t_("asum")
                nc.vector.tensor_tensor(a2t, w2, h2, OP.mult)
                nc.vector.tensor_tensor(a1t, w1, h1, OP.mult)
                eng("asum").tensor_tensor(asum, a1t, a2t, OP.add)

                adx, ady, adW, adH = t_("adx"), t_("ady"), t_("adW"), t_("adH")
                nc.scalar.activation(adx, dx, AF.Abs, scale=2.0)
                nc.scalar.activation(ady, dy, AF.Abs, scale=2.0)
                nc.scalar.activation(adW, dW, AF.Abs)
                nc.scalar.activation(adH, dH, AF.Abs)

                mx, my = t_("mx"), t_("my")
                nc.vector.tensor_tensor(mx, adx, adW, OP.max)
                nc.vector.tensor_tensor(my, ady, adH, OP.max)

                iw4, ih4, ihc, inter4 = t_("iw4"), t_("ih4"), t_("ihc"), t_("inter4")
                nc.vector.scalar_tensor_tensor(iw4, mx, -1.0, W, OP.mult, OP.add)
                nc.vector.scalar_tensor_tensor(ih4, my, -1.0, H, OP.mult, OP.add)
                nc.vector.tensor_scalar(ihc, ih4, 0.0, None, OP.max)
                nc.vector.scalar_tensor_tensor(inter4, iw4, 0.0, ihc, OP.max, OP.mult)

                u = t_("u")
                nc.vector.scalar_tensor_tensor(u, inter4, -0.25, asum, OP.mult, OP.add)
                lnu, r_u = t_("lnu"), t_("r_u")
                nc.scalar.activation(lnu, u, AF.Ln, scale=4.0, bias=bias_ap(4 * EPS))
                nc.scalar.activation(r_u, lnu, AF.Exp, scale=-1.0)
                iou = t_("iou")
                nc.vector.tensor_tensor(iou, inter4, r_u, OP.mult)

                cw2, ch2 = t_("cw2"), t_("ch2")
                eng("cw2").tensor_tensor(cw2, W, mx, OP.add)
                eng("ch2").tensor_tensor(ch2, H, my, OP.add)
                scw, sch, sdx, sdy = t_("scw"), t_("sch"), t_("sdx"), t_("sdy")
                nc.scalar.activation(scw, cw2, AF.Square)
                nc.scalar.activation(sch, ch2, AF.Square)
                nc.scalar.activation(sdx, adx, AF.Square)
                nc.scalar.activation(sdy, ady, AF.Square)
                c24, rho4 = t_("c24"), t_("rho4")
                eng("c24").tensor_tensor(c24, scw, sch, OP.add)
                eng("rho4").tensor_tensor(rho4, sdx, sdy, OP.add)
                lnc, r_c = t_("lnc"), t_("r_c")
                nc.scalar.activation(lnc, c24, AF.Ln, bias=bias_ap(4 * EPS))
                nc.scalar.activation(r_c, lnc, AF.Exp, scale=-1.0)
                term1 = t_("term1")
                nc.vector.tensor_tensor(term1, rho4, r_c, OP.mult)

                # arctan(w/h) for both boxes, range-reduced to [0,1]
                mn2, mxx2, mn1, mxx1 = t_("mn2"), t_("mxx2"), t_("mn1"), t_("mxx1")
                nc.vector.tensor_tensor(mn2, w2, h2, OP.min)
                nc.vector.tensor_tensor(mxx2, w2, h2, OP.max)
                nc.vector.tensor_tensor(mn1, w1, h1, OP.min)
                nc.vector.tensor_tensor(mxx1, w1, h1, OP.max)
                lm2, rr2, lm1, rr1 = t_("lm2"), t_("rr2"), t_("lm1"), t_("rr1")
                nc.scalar.activation(lm2, mxx2, AF.Ln, bias=bias_ap(1e-30))
                nc.scalar.activation(rr2, lm2, AF.Exp, scale=-1.0)
                nc.scalar.activation(lm1, mxx1, AF.Ln, bias=bias_ap(1e-30))
                nc.scalar.activation(rr1, lm1, AF.Exp, scale=-1.0)
                qt2, qt1, sel2, sel1 = t_("qt2"), t_("qt1"), t_("sel2"), t_("sel1")
                nc.vector.tensor_tensor(qt2, mn2, rr2, OP.mult)
                nc.vector.tensor_tensor(qt1, mn1, rr1, OP.mult)
                nc.vector.tensor_tensor(sel2, w2, h2, OP.is_gt)
                nc.vector.tensor_tensor(sel1, w1, h1, OP.is_gt)
                at2, at1 = t_("at2"), t_("at1")
                nc.scalar.activation(at2, qt2, AF.Arctan)
                nc.scalar.activation(at1, qt1, AF.Arctan)
                # theta_i = |sel_i*pi/2 - at_i|  (== atan(w_i/h_i))
                a2d, a1d, th2, th1 = t_("a2d"), t_("a1d"), t_("th2"), t_("th1")
                nc.vector.scalar_tensor_tensor(a2d, sel2, PI / 2, at2, OP.mult, OP.subtract)
                nc.vector.scalar_tensor_tensor(a1d, sel1, PI / 2, at1, OP.mult, OP.subtract)
                nc.scalar.activation(th2, a2d, AF.Abs)
                nc.scalar.activation(th1, a1d, AF.Abs)
                dat = t_("dat")
                eng("dat").tensor_tensor(dat, th2, th1, OP.subtract)
                vv = t_("vv")
                nc.scalar.activation(vv, dat, AF.Square, scale=2.0 / PI)

                den0 = t_("den0")
                nc.vector.tensor_tensor(den0, vv, iou, OP.subtract)
                lnden, rden, v2 = t_("lnden"), t_("rden"), t_("v2")
                nc.scalar.activation(lnden, den0, AF.Ln, bias=bias_ap(1.0 + EPS))
                nc.scalar.activation(rden, lnden, AF.Exp, scale=-1.0)
                nc.scalar.activation(v2, vv, AF.Square)
                term2, s12, z = t_("term2"), t_("s12"), t_("z")
                eng("term2").tensor_tensor(term2, v2, rden, OP.mult)
                eng("s12").tensor_tensor(s12, term1, term2, OP.add)
                nc.vector.scalar_tensor_tensor(z, iou, -1.0, s12, OP.mult, OP.add)

                om2, lnsw, sw = t_("om2"), t_("lnsw"), t_("sw")
                nc.scalar.activation(om2, z, AF.Square, bias=bias_ap(1.0))
                nc.scalar.activation(lnsw, a2t, AF.Ln, bias=bias_ap(1e-7))
                nc.scalar.activation(sw, lnsw, AF.Exp, scale=-1.0)
                om3, baset = t_("om3"), t_("baset")
                nc.vector.scalar_tensor_tensor(om3, z, 1.0, om2, OP.add, OP.mult)
                nc.vector.scalar_tensor_tensor(
                    baset, om3, 0.0, sw, OP.add, OP.mult,
                    accum_out=acc_sb[:, n : n + 1],
                )

                # ---- histogram prep ----
                # floor via magic-number rounding (no mod/divide on DVE ISA):
                # t1 = RNE(32x + 0.5 + 2^23) ; nf = t1 - (2^23+1) = floor(32x)
                # except ties (32x exactly integer k: even k -> k-1) and
                # 32x == 0 -> -1; corrected host-side (see _hist_fix).
                zmx, zmy, q1y = t_("zmx"), t_("zmy"), t_("q1y")
                nfx = tmp.tile([128, T], BF16, tag="nfx", name="nfx")[:]
                nfy = tmp.tile([128, T], BF16, tag="nfy", name="nfy")[:]
                hyb = tmp.tile([128, T], BF16, tag="hyb", name="hyb")[:]
                pyb = tmp.tile([128, T], BF16, tag="pyb", name="pyb")[:]
                wyb = tmp.tile([128, T], BF16, tag="wyb", name="wyb")[:]
                nc.vector.tensor_scalar(zmx, x2, 32.0, 0.5, OP.mult, OP.add)
                nc.vector.tensor_scalar(nfx, zmx, MAGIC, MAGIC + 1.0, OP.add, OP.subtract)
                nc.vector.tensor_scalar(zmy, y2, 32.0, 0.5, OP.mult, OP.add)
                nc.vector.tensor_scalar(nfy, zmy, MAGIC, MAGIC + 1.0, OP.add, OP.subtract)
                # y packed: hy = trickfloor(16y) in [-1..15], py = gy-2hy,
                # wy = 1+511*py in {1,512}; uy[m] = [hy==m]*wy packs bins
                # (2m, 2m+1) into one f32 psum slot (radix 512).
                nc.vector.tensor_scalar(q1y, y2, 16.0, 0.5, OP.mult, OP.add)
                nc.vector.tensor_scalar(hyb, q1y, MAGIC, MAGIC + 1.0, OP.add, OP.subtract)
                nc.vector.scalar_tensor_tensor(pyb, hyb, -2.0, nfy, OP.mult, OP.add)
                nc.vector.tensor_scalar(wyb, pyb, 511.0, 1.0, OP.mult, OP.add)

                for c in range(n_chunks):
                    ohx = ohp.tile([128, GRID * Tc], BF16, tag="ohx", name="ohx")
                    ohy = ohp.tile([128, (GRID // 2) * Tc], BF16, tag="ohy", name="ohy")
                    s = slice(c * Tc, (c + 1) * Tc)
                    for i in range(GRID):
                        nc.vector.tensor_scalar(
                            ohx[:, i * Tc : (i + 1) * Tc], nfx[:, s],
                            float(i), None, OP.is_equal,
                        )
                    for m in range(GRID // 2):
                        nc.vector.scalar_tensor_tensor(
                            ohy[:, m * Tc : (m + 1) * Tc], hyb[:, s],
                            float(m), wyb[:, s], OP.is_equal, OP.mult,
                        )
                    ohx_v = ohx.rearrange("p (i t) -> p t i", t=Tc)
                    ohy_v = ohy.rearrange("p (i t) -> p t i", t=Tc)
                    g = min(n // 2, n_grp - 1)
                    g_mms = (min((2 * g + 2) * 128 * T, NB)) // 128
                    g_first = (2 * g * 128 * T) // 128
                    for t in range(Tc):
                        nc.tensor.matmul(
                            ps_g[g][:], ohy_v[:, t], ohx_v[:, t],
                            start=(mm_i == g_first), stop=(mm_i == g_mms - 1),
                        )
                        mm_i += 1

            for g in range(n_grp):
                nc.vector.tensor_copy(hist_sb[:, g * GRID : (g + 1) * GRID], ps_g[g][:])
            nc.sync.dma_start(hist_d.ap(), hist_sb[:])
            nc.sync.dma_start(acc_d.ap(), acc_sb[:])

    nc.compile()
    return nc


_CACHE = {}
RUN_KW = {}
LAST_RESULT = None


def _get_program(NB, T, Tc):
    key = (NB, T, Tc)
    if key not in _CACHE:
        _CACHE[key] = build_nc(NB, T=T, Tc=Tc)
    return _CACHE[key]


def _trick_bins(v):
    """Replicate the device's magic-number binning exactly (f32 IEEE RNE)."""
    z05 = (v * np.float32(32.0) + np.float32(0.5)).astype(np.float32)  # exact
    t1 = (z05 + np.float32(MAGIC)).astype(np.float32)                  # RNE
    nf = (t1 - np.float32(MAGIC + 1.0)).astype(np.float32)             # exact
    return nf.astype(np.int64)


def _trick16(v):
    z05 = (v * np.float32(16.0) + np.float32(0.5)).astype(np.float32)
    t1 = (z05 + np.float32(MAGIC)).astype(np.float32)
    return (t1 - np.float32(MAGIC + 1.0)).astype(np.float32).astype(np.int64)


def _decode_hists(packed_list, targ, n_shard, pad, T):
    """Decode per-core packed histograms [16, 32*n_grp] (row m packs bins
    2m / 2m+1 at radix 512) into the true 32x32 histogram, moving the few
    fp-tie boxes (where the device trick-bin differs from floor) exactly."""
    n_grp = packed_list[0].shape[1] // GRID
    grp_boxes = 2 * 128 * T
    x, y = targ[:, 0], targ[:, 1]
    gx_t = _trick_bins(x)
    nfy = _trick_bins(y)
    hyb = _trick16(y)
    py = nfy - 2 * hyb
    gx_f = np.floor((x * np.float32(32.0)).astype(np.float32)).astype(np.int64)
    gy_f = np.floor((y * np.float32(32.0)).astype(np.float32)).astype(np.int64)
    clean = (gx_t == gx_f) & (hyb == gy_f // 2) & (py == gy_f % 2)
    hist = np.zeros((GRID, GRID), dtype=np.float64)
    for i in np.nonzero(~clean)[0]:
        c = i // n_shard
        pos = i - c * n_shard
        g = min(pos // grp_boxes, n_grp - 1)
        if 0 <= hyb[i] < 16 and 0 <= gx_t[i] < 32:
            packed_list[c][hyb[i], g * GRID + gx_t[i]] -= 1.0 + 511.0 * py[i]
        hist[gy_f[i], gx_f[i]] += 1.0
    for p in packed_list:
        for g in range(n_grp):
            P = p[:, g * GRID : (g + 1) * GRID]
            n1 = np.floor(P / 512.0)
            n0 = P - 512.0 * n1
            assert (n0 >= 0).all() and (n0 < 512).all() and (n1 >= 0).all(), "decode overflow"
            hist[0::2, :] += n0
            hist[1::2, :] += n1
    if pad:
        # pad box (x=y=0.5): 32v=16 tie->even => bin (15,15)
        hist[15, 15] -= pad * len(packed_list)
    return hist


def kernel(pred_boxes: np.ndarray, target_boxes: np.ndarray) -> np.ndarray:
    N = pred_boxes.shape[0]
    assert N % N_CORES == 0
    n_shard = N // N_CORES
    NB = NB_CORE if N == N_TOTAL else n_shard
    pad = NB - n_shard
    assert pad >= 0

    pred = np.ascontiguousarray(pred_boxes, dtype=np.float32)
    targ = np.ascontiguousarray(target_boxes, dtype=np.float32)

    in_maps = []
    for c in range(N_CORES):
        ps = pred[c * n_shard : (c + 1) * n_shard]
        ts = targ[c * n_shard : (c + 1) * n_shard]
        if pad:
            padrow = np.array(PAD_BOX, dtype=np.float32)[None].repeat(pad, 0)
            ps = np.concatenate([ps, padrow], 0)
            ts = np.concatenate([ts, padrow], 0)
        in_maps.append({"pred_boxes": ps, "target_boxes": ts})

    nc = _get_program(NB, 512, 256)
    res = bass_utils.run_bass_kernel_spmd(
        nc, in_maps, core_ids=list(range(N_CORES)), **RUN_KW
    )
    global LAST_RESULT
    LAST_RESULT = res

    base_sum = 0.0
    packed = []
    for r in res.results:
        base_sum += float(r["acc_out"].astype(np.float64).sum())
        packed.append(r["hist_out"].astype(np.float64))
    hist = _decode_hists(packed, targ, n_shard, pad, 512)
    assert hist.sum() == N, (hist.sum(), N)
    mean_base = base_sum / N
    max_h = hist.max()
    result = mean_base * (1.0 + ALPHA * (N / (GRID * GRID)) / max_h)
    return np.float32(result)

